# revision 6
# baseline (speedup 1.0000x reference)
"""GPT-6L (D=1024, H=16, DFF=4096, V=32000, B=2, T=1024) on 8 trn2 NeuronCores.

Sharding: token-parallel (2048 tokens -> 256/core; cores 0-3 = batch row 0,
cores 4-7 = row 1). Per layer: AllGather of K/V within each row group of 4.
Embedding: vocab-sharded gather + 8-core ReduceScatter. LM head: vocab-sharded
(4000 vocab rows/core), final-x AllGather, host concat. Dense matmuls in fp32r.
Residual kept transposed: xT [D=1024 (8x128 chunks), 256 tokens]. LayerNorm
affines folded into the following weights on host; LN stats via PE ones-matmul
column sums; mean/rstd broadcast across partitions via K=1 matmuls.
"""

import numpy as np

import concourse.bacc as bacc
import concourse.bass as bass
import concourse.tile as tile
from concourse import mybir
from concourse.bass_utils import run_bass_kernel_spmd
from concourse.masks import make_identity

P = 128
NCORES = 8
B, T = 2, 1024
D, DFF, V, NH, HD, L = 1024, 4096, 32000, 16, 64, 6
TL = 256           # tokens per core
DC = D // P        # 8
FC = DFF // P      # 32
VSH = V // NCORES  # 4000
VPAD = 4096
GT = B * T         # 2048
F32 = mybir.dt.float32
F32R = mybir.dt.float32r
I32 = mybir.dt.int32
OOB = 1 << 20
Act = mybir.ActivationFunctionType
Alu = mybir.AluOpType

_CACHE = {}


def _layernorm(nc, sp, bcast_ps, stats_ps, x_tile, h_tile, sq_tile, ones_col, ones_row, eps_tile):
    """h = (x - mean) * rsqrt(var + eps) over the D axis (partition-chunked)."""
    mu_ps = stats_ps.tile([1, TL], F32, tag="mu", space="PSUM")
    ms_ps = stats_ps.tile([1, TL], F32, tag="ms", space="PSUM")
    for dc in range(DC):
        nc.vector.tensor_mul(out=sq_tile[:, dc, :], in0=x_tile[:, dc, :].bitcast(F32),
                             in1=x_tile[:, dc, :].bitcast(F32))
    for dc in range(DC):
        nc.tensor.matmul(mu_ps[:], ones_col[:], x_tile[:, dc, :], start=(dc == 0),
                         stop=(dc == DC - 1), skip_group_check=True)
    for dc in range(DC):
        nc.tensor.matmul(ms_ps[:], ones_col[:], sq_tile[:, dc, :], start=(dc == 0),
                         stop=(dc == DC - 1), skip_group_check=True)
    mu_sb = sp.tile([1, TL], F32R, tag="ln_mu")
    ms_sb = sp.tile([1, TL], F32, tag="ln_ms")
    nc.scalar.activation(out=mu_sb[:], in_=mu_ps[:], func=Act.Identity, scale=1.0 / D)
    nc.scalar.activation(out=ms_sb[:], in_=ms_ps[:], func=Act.Identity, scale=1.0 / D)
    var = sp.tile([1, TL], F32, tag="ln_var")
    nc.vector.tensor_mul(out=var[:], in0=mu_sb[:].bitcast(F32), in1=mu_sb[:].bitcast(F32))
    nc.vector.tensor_tensor(out=var[:], in0=ms_sb[:], in1=var[:], op=Alu.subtract)
    std = sp.tile([1, TL], F32, tag="ln_std")
    nc.scalar.activation(out=std[:], in_=var[:], func=Act.Sqrt, bias=eps_tile[:])
    rstd_f = sp.tile([1, TL], F32, tag="ln_rstdf")
    nc.vector.reciprocal(out=rstd_f[:], in_=std[:])
    rstd = sp.tile([1, TL], F32R, tag="ln_rstd")
    nc.vector.tensor_scalar_mul(out=rstd[:], in0=rstd_f[:], scalar1=1.0)
    mu_bc = bcast_ps.tile([P, TL], F32, tag="mu_bc", space="PSUM")
    rs_bc = bcast_ps.tile([P, TL], F32, tag="rs_bc", space="PSUM")
    nc.tensor.matmul(mu_bc[:], ones_row[:], mu_sb[:], start=True, stop=True)
    nc.tensor.matmul(rs_bc[:], ones_row[:], rstd[:], start=True, stop=True)
    for dc in range(DC):
        nc.vector.tensor_tensor(out=h_tile[:, dc, :], in0=x_tile[:, dc, :].bitcast(F32),
                                in1=mu_bc[:], op=Alu.subtract)
        nc.vector.tensor_tensor(out=h_tile[:, dc, :], in0=h_tile[:, dc, :].bitcast(F32),
                                in1=rs_bc[:], op=Alu.mult)


def _dense_TN(nc, wp, acc_ps, w_ap, rhs_tile, kcn, m_total, evict_fn):
    """psum[mc][P, TL] = sum_kc w_ap[kc*P:(kc+1)*P, mc*P:(mc+1)*P].T @ rhs_tile[:, kc, :]."""
    mcs_all = m_total // P
    for mg in range(0, mcs_all, 4):
        mcs = list(range(mg, min(mg + 4, mcs_all)))
        width = len(mcs) * P
        psums = [acc_ps.tile([P, TL], F32, tag=f"acc{j}", name=f"acc{j}", space="PSUM")
                 for j in range(len(mcs))]
        for kc in range(kcn):
            wt = wp.tile([P, 4 * P], F32R, tag="w")
            nc.sync.dma_start(out=wt[:, :width],
                              in_=w_ap[kc * P:(kc + 1) * P, mg * P:mg * P + width])
            for j in range(len(mcs)):
                nc.tensor.matmul(psums[j][:], wt[:, j * P:(j + 1) * P], rhs_tile[:, kc, :],
                                 start=(kc == 0), stop=(kc == kcn - 1), skip_group_check=True)
        for j, mc in enumerate(mcs):
            evict_fn(mc, psums[j])


def build():
    nc = bacc.Bacc("TRN2", target_bir_lowering=False, debug=False, num_devices=NCORES)

    ids = nc.dram_tensor("ids", [GT, 1], I32, kind="ExternalInput").ap()
    pos = nc.dram_tensor("pos", [TL, D], F32, kind="ExternalInput").ap()
    maskT = nc.dram_tensor("maskT", [T, TL], F32R, kind="ExternalInput").ap()
    embrows = nc.dram_tensor("embrows", [VSH, D], F32, kind="ExternalInput").ap()
    wq = nc.dram_tensor("wq", [L, D, D], F32R, kind="ExternalInput").ap()
    wk = nc.dram_tensor("wk", [L, D, D], F32R, kind="ExternalInput").ap()
    wv = nc.dram_tensor("wv", [L, D, D], F32R, kind="ExternalInput").ap()
    wo = nc.dram_tensor("wo", [L, D, D], F32R, kind="ExternalInput").ap()
    w1 = nc.dram_tensor("w1", [L, D, DFF], F32R, kind="ExternalInput").ap()
    w2 = nc.dram_tensor("w2", [L, DFF, D], F32R, kind="ExternalInput").ap()
    bq = nc.dram_tensor("bq", [L, D], F32, kind="ExternalInput").ap()
    bk = nc.dram_tensor("bk", [L, D], F32, kind="ExternalInput").ap()
    bv = nc.dram_tensor("bv", [L, D], F32, kind="ExternalInput").ap()
    b1 = nc.dram_tensor("b1", [L, DFF], F32, kind="ExternalInput").ap()
    lmw = nc.dram_tensor("lmw", [D, VPAD], F32R, kind="ExternalInput").ap()
    blm = nc.dram_tensor("blm", [VPAD], F32, kind="ExternalInput").ap()
    logits = nc.dram_tensor("logits", [VSH, GT], F32, kind="ExternalOutput").ap()

    with tile.TileContext(nc) as tc:
        with (
            tc.tile_pool(name="const", bufs=1) as const,
            tc.tile_pool(name="persist", bufs=1) as persist,
            tc.tile_pool(name="sp", bufs=2) as sp,
            tc.tile_pool(name="wp", bufs=3) as wp,
            tc.tile_pool(name="wvp", bufs=2) as wvp,
            tc.tile_pool(name="dram", bufs=2, space="DRAM") as dram,
        ):
            ident = const.tile([P, P], F32)
            make_identity(nc, ident)
            ones_f = const.tile([P, 1], F32)
            nc.vector.memset(ones_f[:], 1.0)
            ones_fr = const.tile([1, P], F32)
            nc.vector.memset(ones_fr[:], 1.0)
            ones_col = const.tile([P, 1], F32R)
            nc.vector.tensor_scalar_mul(out=ones_col[:], in0=ones_f[:], scalar1=1.0)
            ones_row = const.tile([1, P], F32R)
            nc.vector.tensor_scalar_mul(out=ones_row[:], in0=ones_fr[:], scalar1=1.0)
            ones_head = const.tile([1, HD], F32R)
            nc.vector.tensor_scalar_mul(out=ones_head[:], in0=ones_fr[:, :HD], scalar1=1.0)
            eps_tile = const.tile([1, 1], F32)
            nc.vector.memset(eps_tile[:], 1e-5)
            mask_sb = const.tile([P, DC, TL], F32R)
            nc.sync.dma_start(out=mask_sb[:], in_=maskT.rearrange("(kc p) q -> p kc q", p=P))

            xT = persist.tile([P, DC, TL], F32R)

            # ---------------- embedding ----------------
            with (
                tc.tile_pool(name="emb", bufs=1) as emb,
                tc.tile_pool(name="tr_ps", bufs=4, space="PSUM") as tr_ps,
            ):
                ids_sb = emb.tile([P, GT // P], I32)
                nc.sync.dma_start(out=ids_sb[:], in_=ids.rearrange("(g p) one -> p (g one)", p=P))
                x0 = emb.tile([P, GT // P, D], F32)
                nc.vector.memset(x0[:], 0.0)
                for g in range(GT // P):
                    nc.gpsimd.indirect_dma_start(
                        out=x0[:, g, :], out_offset=None, in_=embrows[:],
                        in_offset=bass.IndirectOffsetOnAxis(ap=ids_sb[:, g:g + 1], axis=0),
                        bounds_check=VSH - 1, oob_is_err=False)
                rs_in = dram.tile([GT, D], F32, tag="rs_in")
                rs_out = dram.tile([TL, D], F32, tag="rs_out")
                nc.sync.dma_start(out=rs_in[:].rearrange("(g p) d -> p g d", p=P), in_=x0[:])
                nc.gpsimd.collective_compute(
                    "ReduceScatter", Alu.add, replica_groups=[list(range(NCORES))],
                    ins=[rs_in.opt()], outs=[rs_out.opt()])
                x_tm = emb.tile([P, TL // P, D], F32)
                nc.sync.dma_start(out=x_tm[:], in_=rs_out[:].rearrange("(tc p) d -> p tc d", p=P))
                pos_sb = emb.tile([P, TL // P, D], F32)
                nc.sync.dma_start(out=pos_sb[:], in_=pos.rearrange("(tc p) d -> p tc d", p=P))
                nc.vector.tensor_add(out=x_tm[:], in0=x_tm[:], in1=pos_sb[:])
                for dc in range(DC):
                    for tcn in range(TL // P):
                        pt = tr_ps.tile([P, P], F32, tag="tr", space="PSUM")
                        nc.tensor.transpose(pt[:], x_tm[:, tcn, dc * P:(dc + 1) * P], ident[:])
                        nc.scalar.copy(out=xT[:, dc, tcn * P:(tcn + 1) * P], in_=pt[:])

            # ---------------- transformer layers ----------------
            with tc.tile_pool(name="lp", bufs=1) as lp:
                hT = lp.tile([P, DC, TL], F32R)
                sq = lp.tile([P, DC, TL], F32R)
                qT = lp.tile([P, DC, TL], F32R)
                kTl = lp.tile([P, DC, TL], F32R)
                vL = lp.tile([P, TL // P, D], F32R)
                kF = lp.tile([P, DC, T], F32R)
                vF = lp.tile([P, DC, T], F32R)
                oT = lp.tile([P, DC, TL], F32R)
                aT = lp.tile([P, FC, TL], F32R)

                for l in range(L):
                    with (
                        tc.tile_pool(name=f"st{l}a", bufs=1, space="PSUM") as stats_ps,
                        tc.tile_pool(name=f"bc{l}a", bufs=1, space="PSUM") as bcast_ps,
                    ):
                        _layernorm(nc, sp, bcast_ps, stats_ps, xT, hT, sq,
                                   ones_col, ones_row, eps_tile)

                    bq_t = sp.tile([P, DC], F32, tag="bq")
                    nc.sync.dma_start(out=bq_t[:], in_=bq[l].rearrange("(c p) -> p c", p=P))
                    bk_t = sp.tile([P, DC], F32, tag="bk")
                    nc.sync.dma_start(out=bk_t[:], in_=bk[l].rearrange("(c p) -> p c", p=P))
                    bv_t = sp.tile([P, DC], F32, tag="bv")
                    nc.sync.dma_start(out=bv_t[:], in_=bv[l].rearrange("(c p) -> p c", p=P))

                    with tc.tile_pool(name=f"qk{l}", bufs=1, space="PSUM") as acc_ps:
                        # K first so its AllGather launches asap
                        def ev_k(mc, ps):
                            nc.scalar.activation(out=kTl[:, mc, :], in_=ps[:], func=Act.Identity,
                                                 bias=bk_t[:, mc:mc + 1])
                        _dense_TN(nc, wp, acc_ps, wk[l], hT, DC, D, ev_k)
                        kag_in = dram.tile([D, TL], F32R, tag="kag_in")
                        kag_out = dram.tile([4, D, TL], F32R, tag="kag_out")
                        nc.sync.dma_start(out=kag_in[:].rearrange("(c p) t -> p c t", p=P),
                                          in_=kTl[:])
                        nc.gpsimd.collective_compute(
                            "AllGather", Alu.bypass,
                            replica_groups=[[0, 1, 2, 3], [4, 5, 6, 7]],
                            ins=[kag_in.opt()], outs=[kag_out.opt()])

                        # V projection: out[tokens, vdim] = h @ Wv^T (lhsT = hT chunks)
                        pvs = [[acc_ps.tile([P, 512], F32, tag=f"vacc{mc}{nv}", name=f"vacc{mc}{nv}",
                                            space="PSUM")
                                for nv in range(2)] for mc in range(TL // P)]
                        for kc in range(DC):
                            wt = wvp.tile([P, D], F32R, tag="wv")
                            nc.sync.dma_start(out=wt[:], in_=wv[l][kc * P:(kc + 1) * P, :])
                            for mc in range(TL // P):
                                for nv in range(2):
                                    nc.tensor.matmul(
                                        pvs[mc][nv][:], hT[:, kc, mc * P:(mc + 1) * P],
                                        wt[:, nv * 512:(nv + 1) * 512],
                                        start=(kc == 0), stop=(kc == DC - 1),
                                        skip_group_check=True)
                        for mc in range(TL // P):
                            for nv in range(2):
                                nc.scalar.copy(out=vL[:, mc, nv * 512:(nv + 1) * 512],
                                               in_=pvs[mc][nv][:])
                        vag_in = dram.tile([TL, D], F32R, tag="vag_in")
                        vag_out = dram.tile([4, TL, D], F32R, tag="vag_out")
                        nc.sync.dma_start(out=vag_in[:].rearrange("(c p) d -> p c d", p=P),
                                          in_=vL[:])
                        nc.gpsimd.collective_compute(
                            "AllGather", Alu.bypass,
                            replica_groups=[[0, 1, 2, 3], [4, 5, 6, 7]],
                            ins=[vag_in.opt()], outs=[vag_out.opt()])

                        def ev_q(mc, ps):
                            nc.scalar.activation(out=qT[:, mc, :], in_=ps[:], func=Act.Identity,
                                                 bias=bq_t[:, mc:mc + 1])
                        _dense_TN(nc, wp, acc_ps, wq[l], hT, DC, D, ev_q)

                        for dc in range(DC):
                            nc.sync.dma_start(
                                out=kF[:, dc, :].rearrange("p (r t) -> p r t", r=4),
                                in_=kag_out[:, dc * P:(dc + 1) * P, :].rearrange(
                                    "r p t -> p r t"))
                        for kc in range(DC):
                            nc.sync.dma_start(
                                out=vF[:, kc, :],
                                in_=vag_out[kc // 2, (kc % 2) * P:(kc % 2) * P + P, :])

                    # ---------------- attention ----------------
                    with (
                        tc.tile_pool(name=f"sc{l}", bufs=3, space="PSUM") as sc_ps,
                        tc.tile_pool(name=f"rs{l}", bufs=2, space="PSUM") as rsum_ps,
                        tc.tile_pool(name=f"av{l}", bufs=2, space="PSUM") as av_ps,
                        tc.tile_pool(name=f"rbc{l}", bufs=1, space="PSUM") as rbc_ps,
                        tc.tile_pool(name=f"ex{l}", bufs=3) as ex_sp,
                    ):
                        for h in range(NH):
                            po = (h % 2) * HD
                            hc = h // 2
                            rsum = rsum_ps.tile([1, TL], F32, tag="rsum", space="PSUM")
                            avp = av_ps.tile([HD, TL], F32, tag="av", space="PSUM")
                            for kc in range(DC):
                                scp = sc_ps.tile([P, TL], F32, tag="sc", space="PSUM")
                                nc.tensor.matmul(scp[:],
                                                 kF[po:po + HD, hc, kc * P:(kc + 1) * P],
                                                 qT[po:po + HD, hc, :], start=True, stop=True,
                                                 skip_group_check=True)
                                ex = ex_sp.tile([P, TL], F32R, tag="ex")
                                nc.scalar.activation(out=ex[:], in_=scp[:], func=Act.Exp,
                                                     scale=0.125)
                                nc.vector.tensor_tensor(out=ex[:], in0=ex[:].bitcast(F32),
                                                        in1=mask_sb[:, kc, :].bitcast(F32),
                                                        op=Alu.mult)
                                nc.tensor.matmul(rsum[:], ones_col[:], ex[:],
                                                 start=(kc == 0), stop=(kc == DC - 1),
                                                 skip_group_check=True)
                                nc.tensor.matmul(avp[:], vF[:, kc, h * HD:(h + 1) * HD], ex[:],
                                                 start=(kc == 0), stop=(kc == DC - 1),
                                                 skip_group_check=True)
                            rcp_f = sp.tile([1, TL], F32, tag="rcp_f")
                            nc.vector.reciprocal(out=rcp_f[:], in_=rsum[:])
                            rcp = sp.tile([1, TL], F32R, tag="rcp")
                            nc.vector.tensor_scalar_mul(out=rcp[:], in0=rcp_f[:], scalar1=1.0)
                            rbc = rbc_ps.tile([HD, TL], F32, tag="rbc", space="PSUM")
                            nc.tensor.matmul(rbc[:], ones_head[:], rcp[:], start=True, stop=True,
                                             skip_group_check=True)
                            rbc_sb = ex_sp.tile([HD, TL], F32, tag="rbc_sb", bufs=2)
                            nc.scalar.copy(out=rbc_sb[:], in_=rbc[:])
                            nc.vector.tensor_tensor(out=oT[po:po + HD, hc, :], in0=avp[:],
                                                    in1=rbc_sb[:], op=Alu.mult)
                            nc.vector.tensor_scalar_add(
                                out=oT[po:po + HD, hc, :],
                                in0=oT[po:po + HD, hc, :].bitcast(F32),
                                scalar1=bv_t[po:po + HD, hc:hc + 1])

                    with tc.tile_pool(name=f"op{l}", bufs=2, space="PSUM") as acc_ps:
                        def ev_o(mc, ps):
                            nc.vector.tensor_tensor(out=xT[:, mc, :],
                                                    in0=xT[:, mc, :].bitcast(F32),
                                                    in1=ps[:], op=Alu.add)
                        _dense_TN(nc, wp, acc_ps, wo[l], oT, DC, D, ev_o)

                    with (
                        tc.tile_pool(name=f"st{l}b", bufs=1, space="PSUM") as stats_ps,
                        tc.tile_pool(name=f"bc{l}b", bufs=1, space="PSUM") as bcast_ps,
                    ):
                        _layernorm(nc, sp, bcast_ps, stats_ps, xT, hT, sq,
                                   ones_col, ones_row, eps_tile)

                    b1_t = sp.tile([P, FC], F32, tag="b1")
                    nc.sync.dma_start(out=b1_t[:], in_=b1[l].rearrange("(c p) -> p c", p=P))
                    with tc.tile_pool(name=f"f1{l}", bufs=2, space="PSUM") as acc_ps:
                        def ev_f1(mc, ps):
                            nc.scalar.activation(out=aT[:, mc, :], in_=ps[:], func=Act.Gelu,
                                                 bias=b1_t[:, mc:mc + 1])
                        _dense_TN(nc, wp, acc_ps, w1[l], hT, DC, DFF, ev_f1)
                    with tc.tile_pool(name=f"f2{l}", bufs=2, space="PSUM") as acc_ps:
                        def ev_f2(mc, ps):
                            nc.vector.tensor_tensor(out=xT[:, mc, :],
                                                    in0=xT[:, mc, :].bitcast(F32),
                                                    in1=ps[:], op=Alu.add)
                        _dense_TN(nc, wp, acc_ps, w2[l], aT, FC, D, ev_f2)

                # final LN (into hT) + x AllGather — inside lp scope
                with (
                    tc.tile_pool(name="stf", bufs=1, space="PSUM") as stats_ps,
                    tc.tile_pool(name="bcf", bufs=1, space="PSUM") as bcast_ps,
                ):
                    _layernorm(nc, sp, bcast_ps, stats_ps, xT, hT, sq,
                               ones_col, ones_row, eps_tile)
                xag_in = dram.tile([D, TL], F32R, tag="xag_in")
                xag_out = dram.tile([NCORES, D, TL], F32R, tag="xag_out")
                nc.sync.dma_start(out=xag_in[:].rearrange("(c p) t -> p c t", p=P), in_=hT[:])
                nc.gpsimd.collective_compute(
                    "AllGather", Alu.bypass, replica_groups=[list(range(NCORES))],
                    ins=[xag_in.opt()], outs=[xag_out.opt()])

            # ---------------- LM head ----------------
            with (
                tc.tile_pool(name="lm", bufs=1) as lmp,
                tc.tile_pool(name="lmw", bufs=2) as lmwp,
                tc.tile_pool(name="lmacc", bufs=2, space="PSUM") as acc_ps,
                tc.tile_pool(name="lmev", bufs=4) as evp,
            ):
                xfT = lmp.tile([P, DC, GT], F32R)
                for dc in range(DC):
                    nc.sync.dma_start(
                        out=xfT[:, dc, :].rearrange("p (r t) -> p r t", r=NCORES),
                        in_=xag_out[:, dc * P:(dc + 1) * P, :].rearrange("r p t -> p r t"))
                blm_t = lmp.tile([P, VPAD // P], F32)
                nc.sync.dma_start(out=blm_t[:], in_=blm.rearrange("(c p) -> p c", p=P))
                for mg in range(0, VPAD // P, 4):
                    wts = []
                    for kc in range(DC):
                        wt = lmwp.tile([P, 4 * P], F32R, tag=f"lw{kc}")
                        nc.sync.dma_start(out=wt[:],
                                          in_=lmw[kc * P:(kc + 1) * P, mg * P:(mg + 4) * P])
                        wts.append(wt)
                    for ncn in range(GT // TL):
                        psums = [acc_ps.tile([P, TL], F32, tag=f"acc{j}", name=f"lmacc{j}", space="PSUM")
                                 for j in range(4)]
                        for kc in range(DC):
                            for j in range(4):
                                nc.tensor.matmul(psums[j][:], wts[kc][:, j * P:(j + 1) * P],
                                                 xfT[:, kc, ncn * TL:(ncn + 1) * TL],
                                                 start=(kc == 0), stop=(kc == DC - 1),
                                                 skip_group_check=True)
                        for j in range(4):
                            mc = mg + j
                            rows = min(P, VSH - mc * P)
                            if rows <= 0:
                                continue
                            res = evp.tile([P, TL], F32, tag="res")
                            nc.scalar.activation(out=res[:], in_=psums[j][:], func=Act.Identity,
                                                 bias=blm_t[:, mc:mc + 1])
                            nc.sync.dma_start(
                                out=logits[mc * P:mc * P + rows, ncn * TL:(ncn + 1) * TL],
                                in_=res[:rows, :])

    nc.compile()
    return nc


def _prep_inputs(inputs):
    idx = np.asarray(inputs["idx"], dtype=np.int32).reshape(-1)
    tok_emb = np.asarray(inputs["tok_emb"], dtype=np.float32)
    pos_emb = np.asarray(inputs["pos_emb"], dtype=np.float32)
    qkv_w = np.asarray(inputs["qkv_w"], dtype=np.float32)
    out_w = np.asarray(inputs["out_w"], dtype=np.float32)
    ffn1_w = np.asarray(inputs["ffn1_w"], dtype=np.float32)
    ffn2_w = np.asarray(inputs["ffn2_w"], dtype=np.float32)
    ln1_w = np.asarray(inputs["ln1_w"], dtype=np.float32)
    ln1_b = np.asarray(inputs["ln1_b"], dtype=np.float32)
    ln2_w = np.asarray(inputs["ln2_w"], dtype=np.float32)
    ln2_b = np.asarray(inputs["ln2_b"], dtype=np.float32)
    lnf_w = np.asarray(inputs["lnf_w"], dtype=np.float32)
    lnf_b = np.asarray(inputs["lnf_b"], dtype=np.float32)

    wq = np.empty((L, D, D), np.float32); wk = np.empty((L, D, D), np.float32)
    wv = np.empty((L, D, D), np.float32); wo = np.empty((L, D, D), np.float32)
    w1 = np.empty((L, D, DFF), np.float32); w2 = np.empty((L, DFF, D), np.float32)
    bq = np.empty((L, D), np.float32); bk = np.empty((L, D), np.float32)
    bv = np.empty((L, D), np.float32); b1 = np.empty((L, DFF), np.float32)
    for l in range(L):
        Wq, Wk, Wv = qkv_w[l, :D], qkv_w[l, D:2 * D], qkv_w[l, 2 * D:]
        wq[l] = Wq.T * ln1_w[l][:, None]
        wk[l] = Wk.T * ln1_w[l][:, None]
        wv[l] = Wv.T * ln1_w[l][:, None]
        bq[l] = ln1_b[l] @ Wq.T
        bk[l] = ln1_b[l] @ Wk.T
        bv[l] = ln1_b[l] @ Wv.T
        wo[l] = out_w[l].T
        w1[l] = ffn1_w[l].T * ln2_w[l][:, None]
        b1[l] = ln2_b[l] @ ffn1_w[l].T
        w2[l] = ffn2_w[l].T

    lmw_full = tok_emb.T * lnf_w[:, None]
    blm_full = tok_emb @ lnf_b

    in_maps = []
    for c in range(NCORES):
        blk = c % 4
        ids_shift = np.where((idx >= c * VSH) & (idx < (c + 1) * VSH), idx - c * VSH, OOB)
        ids_shift = ids_shift.astype(np.int32).reshape(GT, 1)
        q_glob = blk * TL + np.arange(TL)
        maskT_np = (np.arange(T)[:, None] <= q_glob[None, :]).astype(np.float32)
        lmw_c = np.zeros((D, VPAD), np.float32)
        lmw_c[:, :VSH] = lmw_full[:, c * VSH:(c + 1) * VSH]
        blm_c = np.zeros((VPAD,), np.float32)
        blm_c[:VSH] = blm_full[c * VSH:(c + 1) * VSH]
        in_maps.append({
            "ids": ids_shift,
            "pos": np.ascontiguousarray(pos_emb[blk * TL:(blk + 1) * TL]),
            "maskT": maskT_np,
            "embrows": np.ascontiguousarray(tok_emb[c * VSH:(c + 1) * VSH]),
            "wq": np.ascontiguousarray(wq), "wk": np.ascontiguousarray(wk),
            "wv": np.ascontiguousarray(wv), "wo": np.ascontiguousarray(wo),
            "w1": np.ascontiguousarray(w1), "w2": np.ascontiguousarray(w2),
            "bq": bq, "bk": bk, "bv": bv, "b1": b1,
            "lmw": lmw_c, "blm": blm_c,
        })
    return in_maps


def kernel(**inputs):
    in_maps = _prep_inputs(inputs)
    if "nc" not in _CACHE:
        _CACHE["nc"] = build()
    res = run_bass_kernel_spmd(_CACHE["nc"], in_maps, core_ids=list(range(NCORES)))
    parts = [res.results[c]["logits"] for c in range(NCORES)]
    full = np.concatenate(parts, axis=0)          # [V, GT]
    return np.ascontiguousarray(full.T).reshape(B, T, V)
